# revision 19
# baseline (speedup 1.0000x reference)
"""LIF layer (leaky integrate-and-fire scan over time) on 8 Trainium2 cores.

Recurrence per (b, f) row over t = 0..L-1 (reference semantics):
    v_pre[t] = alpha[f] * v[t-1] + (1 - alpha[f]) * I[b, f, t]
    z[t]     = BETA * (v_pre[t] - THR)
    s[t]     = (v_pre[t] >= THR)
    v[t]     = v_pre[t] * (v_pre[t] < THR)          # reset on spike

Outputs: (v_pre, z, s) each [B, F, L] float32.

Device algorithm (v5):
- Scaled state u = v / (1 - alpha) turns the step into u' = alpha*u + I[t]
  (raw input feeds the chain directly, no J=(1-alpha)*I precompute) with the
  reset compare against a per-partition threshold theta = THR/(1-alpha):
      STT1: u_pre = (u * alpha) + I[t]
      STT2: u     = (u_pre < theta) * u_pre
- Sharding: 8 cores = 2 f-halves x 4 time-quarters of 512 steps. Within a
  quarter the serial scan is split across engines and segments:
    DVE:  3 segments of 144 steps, fused into one op pair (free = 3 segs).
    Pool: 1 segment of 80 steps (no scalar_tensor_tensor in the Pool ISA;
          uses the 3-op form m=(u<theta)*alpha; g=u*m; u'=g+I).
  Each segment re-converges state with a 48-step warmup (decay alpha^48 is
  below fp32 resolution for this alpha range; spike resets heal the rest).
- Latency hiding: each engine runs TWO interleaved chain groups over the two
  batch halves (b 0:32 / 32:64). Consecutive ops of one group are 2 issue
  slots apart, which covers the same-engine semaphore dependency latency, so
  the engines run at their raw per-op rate.
- Outputs on ScalarE from the f32 u_pre chunks:
      v = bf16(omalpha * u_pre)                    (Copy, per-partition scale)
      z = bf16(15*omalpha * u_pre - 3.75)          (Copy, scale + const bias)
      s = fp8(sigmoid(1e30 * u_pre - 1e30*theta))  (saturates to exact 0/1)
  Host upcasts to f32; norm rel-err from bf16 is ~1.7e-3, inside 2e-2.
"""

import sys

sys.path.insert(0, "/opt/trn_rl_repo")

import numpy as np

DT = 1.0
BETA = 15.0
THR = 0.25

B, F, L = 64, 256, 2048
N_CORES = 8
FB = 128          # partition block of F per core
NQ = 4            # time quarters
QLEN = L // NQ    # 512
TC = 16           # time-chunk length
W = 32            # warmup steps per segment
KD = 4            # DVE fused segment count
TD = 112          # DVE segment length
TP = 64           # Pool segment length
HB = 32           # half-batch for the two interleaved chain groups
SIGM = 1.0e30     # sigmoid saturation scale for the spike output

NDC = (W + TD) // TC   # 12 DVE chunks (3 warmup + 9 output)
NPC = (W + TP) // TC   # 8 Pool chunks (3 warmup + 5 output)
NWC = W // TC          # 3

_BUILD_CACHE: dict = {}
LAST_RESULTS = None  # BassKernelResults of the most recent kernel() call
_CURRENT_NC = None


def _get_current_nc():
    return _CURRENT_NC


def _build_v5():
    import concourse.bacc as bacc
    import concourse.mybir as mybir
    from concourse import tile

    f32 = mybir.dt.float32
    bf16 = mybir.dt.bfloat16
    fp8 = mybir.dt.float8e4
    Alu = mybir.AluOpType
    Act = mybir.ActivationFunctionType

    nc = bacc.Bacc(None, target_bir_lowering=False)

    iD = nc.dram_tensor("i_dve", [128, NDC, 2, TC, KD, HB], f32, kind="ExternalInput")
    iP = nc.dram_tensor("i_pool", [128, NPC, 2, TC, HB], f32, kind="ExternalInput")
    cst_d = nc.dram_tensor("consts", [128, 5], f32, kind="ExternalInput")

    nod, nop = NDC - NWC, NPC - NWC
    vD = nc.dram_tensor("v_dve", [128, nod, 2, TC, KD, HB], bf16, kind="ExternalOutput")
    zD = nc.dram_tensor("z_dve", [128, nod, 2, TC, KD, HB], bf16, kind="ExternalOutput")
    sD = nc.dram_tensor("s_dve", [128, nod, 2, TC, KD, HB], fp8, kind="ExternalOutput")
    vP = nc.dram_tensor("v_pool", [128, nop, 2, TC, HB], bf16, kind="ExternalOutput")
    zP = nc.dram_tensor("z_pool", [128, nop, 2, TC, HB], bf16, kind="ExternalOutput")
    sP = nc.dram_tensor("s_pool", [128, nop, 2, TC, HB], fp8, kind="ExternalOutput")

    with tile.TileContext(nc) as tc_:
        with (
            tc_.tile_pool(name="const", bufs=1) as cp,
            tc_.tile_pool(name="din", bufs=3) as din,
            tc_.tile_pool(name="dwork", bufs=4) as dw,
            tc_.tile_pool(name="dout", bufs=2) as do_,
            tc_.tile_pool(name="pin", bufs=6) as pin,
            tc_.tile_pool(name="pwork", bufs=2) as pw,
            tc_.tile_pool(name="pout", bufs=2) as po,
        ):
            # first input chunks, split fine so the chains can start early;
            # consts ride the ACT ring in one packed transfer.
            it0 = din.tile([128, 2, TC, KD, HB], f32, tag="di")
            for h in (0, 1):
                nc.sync.dma_start(it0[:, h, 0 : TC // 2], iD[:, 0, h, 0 : TC // 2])
            ip0 = pin.tile([128, 2, TC, HB], f32, tag="pi")
            for h in (0, 1):
                nc.sync.dma_start(ip0[:, h, 0 : TC // 2], iP[:, 0, h, 0 : TC // 2])
            for h in (0, 1):
                nc.sync.dma_start(it0[:, h, TC // 2 :], iD[:, 0, h, TC // 2 :])
                nc.sync.dma_start(ip0[:, h, TC // 2 :], iP[:, 0, h, TC // 2 :])

            cst_t = cp.tile([128, 5], f32, tag="cst")
            nc.scalar.dma_start(cst_t[:], cst_d[:])
            al_t = cst_t[:, 0:1]
            th_t = cst_t[:, 1:2]
            sv_t = cst_t[:, 2:3]
            sz_t = cst_t[:, 3:4]
            sb_t = cst_t[:, 4:5]

            ustD = cp.tile([128, 2, KD, HB], f32, tag="ustD")
            upwD = cp.tile([128, 2, KD, HB], f32, tag="upwD")
            nc.gpsimd.memset(ustD[:], 0.0)
            # Pool state holds u_pre (pre-reset); m/g are per-half scratch.
            upwP = cp.tile([128, 2, HB], f32, tag="upwP")
            mP = cp.tile([128, 2, HB], f32, tag="mP")
            gP = cp.tile([128, 2, HB], f32, tag="gP")
            nc.gpsimd.memset(upwP[:], 0.0)
            uprevP = [upwP[:, 0], upwP[:, 1]]
            pending_pout = []

            for k in range(NDC):
                # ---- DVE: KD fused segments, 2 interleaved b-half groups ----
                if k == 0:
                    it = it0
                else:
                    it = din.tile([128, 2, TC, KD, HB], f32, tag="di")
                    nc.sync.dma_start(it[:], iD[:, k])
                is_out = k >= NWC
                up = None
                if is_out:
                    up = dw.tile([128, 2, TC, KD, HB], f32, tag="dup")
                for t in range(TC):
                    if is_out:
                        dsts = (up[:, 0, t], up[:, 1, t])
                    else:
                        dsts = (upwD[:, 0], upwD[:, 1])
                    for h in (0, 1):
                        nc.vector.scalar_tensor_tensor(
                            dsts[h], ustD[:, h], al_t, it[:, h, t],
                            op0=Alu.mult, op1=Alu.add,
                        )
                    for h in (0, 1):
                        nc.vector.scalar_tensor_tensor(
                            ustD[:, h], dsts[h], th_t, dsts[h],
                            op0=Alu.is_lt, op1=Alu.mult,
                        )
                # deferred pool-chunk v/s on DVE (pool chunk k-1 is done)
                if pending_pout:
                    po_, pupp = pending_pout.pop(0)
                    pvt = po.tile([128, 2, TC, HB], bf16, tag="pv")
                    nc.scalar.activation(pvt[:], pupp[:], Act.Copy, bias=0.0, scale=sv_t)
                    pst = po.tile([128, 2, TC, HB], fp8, tag="ps")
                    nc.scalar.activation(pst[:], pupp[:], Act.Sigmoid, bias=sb_t, scale=SIGM)
                    pzt = po.tile([128, 2, TC, HB], bf16, tag="pz")
                    nc.scalar.activation(pzt[:], pupp[:], Act.Copy, bias=-BETA * THR, scale=sz_t)
                    nc.scalar.dma_start(vP[:, po_], pvt[:])
                    nc.scalar.dma_start(zP[:, po_], pzt[:])
                    nc.scalar.dma_start(sP[:, po_], pst[:])

                if is_out:
                    o = k - NWC
                    vt = do_.tile([128, 2, TC, KD, HB], bf16, tag="dv")
                    zt = do_.tile([128, 2, TC, KD, HB], bf16, tag="dz")
                    st = do_.tile([128, 2, TC, KD, HB], fp8, tag="ds")
                    if k == NDC - 1:
                        # tail chunk: chain engines are free; keep ACT off the
                        # critical tail
                        nc.vector.tensor_scalar(vt[:], up[:], sv_t, None, Alu.mult)
                        nc.vector.tensor_scalar(st[:], up[:], th_t, None, Alu.is_ge)
                    elif k >= NPC - 1:
                        # Pool chain is done by now; use it for v/s
                        nc.gpsimd.tensor_scalar(vt[:], up[:], sv_t, None, Alu.mult)
                        nc.gpsimd.tensor_scalar(st[:], up[:], th_t, None, Alu.is_ge)
                    else:
                        nc.scalar.activation(vt[:], up[:], Act.Copy, bias=0.0, scale=sv_t)
                        nc.scalar.activation(st[:], up[:], Act.Sigmoid, bias=sb_t, scale=SIGM)
                    nc.scalar.activation(zt[:], up[:], Act.Copy, bias=-BETA * THR, scale=sz_t)
                    nc.scalar.dma_start(vD[:, o], vt[:])
                    nc.scalar.dma_start(zD[:, o], zt[:])
                    nc.scalar.dma_start(sD[:, o], st[:])

                # ---- Pool: 1 segment, 3-op form, 2 interleaved b-half groups ----
                if k >= NPC:
                    continue
                if k == 0:
                    ip = ip0
                else:
                    ip = pin.tile([128, 2, TC, HB], f32, tag="pi")
                    nc.sync.dma_start(ip[:], iP[:, k])
                is_out = k >= NWC
                upp = None
                if is_out:
                    upp = pw.tile([128, 2, TC, HB], f32, tag="pup")
                for t in range(TC):
                    if is_out:
                        dsts = (upp[:, 0, t], upp[:, 1, t])
                    else:
                        dsts = (upwP[:, 0], upwP[:, 1])
                    for h in (0, 1):
                        nc.gpsimd.tensor_scalar(
                            mP[:, h], uprevP[h], th_t, al_t,
                            Alu.is_lt, Alu.mult,
                        )
                    for h in (0, 1):
                        nc.gpsimd.tensor_mul(gP[:, h], uprevP[h], mP[:, h])
                    for h in (0, 1):
                        nc.gpsimd.tensor_add(dsts[h], gP[:, h], ip[:, h, t])
                    uprevP = [dsts[0], dsts[1]]
                if is_out:
                    pending_pout.append((k - NWC, upp))

            # flush remaining pool outputs (after all chains)
            for po_, pupp in pending_pout:
                pvt = po.tile([128, 2, TC, HB], bf16, tag="pv")
                nc.scalar.activation(pvt[:], pupp[:], Act.Copy, bias=0.0, scale=sv_t)
                pst = po.tile([128, 2, TC, HB], fp8, tag="ps")
                nc.scalar.activation(pst[:], pupp[:], Act.Sigmoid, bias=sb_t, scale=SIGM)
                pzt = po.tile([128, 2, TC, HB], bf16, tag="pz")
                nc.scalar.activation(pzt[:], pupp[:], Act.Copy, bias=-BETA * THR, scale=sz_t)
                nc.scalar.dma_start(vP[:, po_], pvt[:])
                nc.scalar.dma_start(zP[:, po_], pzt[:])
                nc.scalar.dma_start(sP[:, po_], pst[:])

    nc.compile()
    return nc


def _get_nc():
    key = ("v5", TC, W, KD, TD, TP)
    if key not in _BUILD_CACHE:
        _BUILD_CACHE[key] = _build_v5()
    return _BUILD_CACHE[key]


# --------------------------------------------------------------------------
# Host side


def _alpha_host(raw_tau: np.ndarray) -> tuple[np.ndarray, np.ndarray]:
    """alpha = exp(-DT / (softplus(raw_tau) + 1e-4)) with the same jax ops /
    device as the reference, so spike threshold comparisons match closely."""
    import jax
    import jax.numpy as jnp

    with jax.default_device(jax.devices("cpu")[0]):
        tau = jax.nn.softplus(jnp.asarray(np.asarray(raw_tau))) + 1e-4
        alpha = np.asarray(jnp.exp(-DT / tau), dtype=np.float32)
    one_minus = (np.float32(1.0) - alpha).astype(np.float32)
    return alpha, one_minus


def _pack_core(IT: np.ndarray, q: int) -> tuple[np.ndarray, np.ndarray]:
    """IT: [128, L, 64] (f, t, b) for this core's f-block. Returns the DVE
    and Pool input streams with warmup prefixes (zero-padded below t=0)."""
    r0 = q * QLEN
    tD = np.empty((W + TD, KD), np.int64)
    for j in range(KD):
        start = r0 + j * TD
        tD[:, j] = np.arange(start - W, start + TD)
    tP = np.arange(r0 + KD * TD - W, r0 + QLEN)

    mD = tD >= 0
    mP = tP >= 0
    iD = IT[:, np.clip(tD, 0, L - 1), :]        # [128, W+TD, KD, 64]
    iD[:, ~mD] = 0.0
    iP = IT[:, np.clip(tP, 0, L - 1), :]        # [128, W+TP, 64]
    iP[:, ~mP] = 0.0
    # device layout: [128, chunk, h(2), t(TC), KD, HB] with b = h*HB + b'
    iD = iD.reshape(128, NDC, TC, KD, 2, HB).transpose(0, 1, 4, 2, 3, 5)
    iP = iP.reshape(128, NPC, TC, 2, HB).transpose(0, 1, 3, 2, 4)
    return np.ascontiguousarray(iD), np.ascontiguousarray(iP)


def kernel(I: np.ndarray, raw_tau: np.ndarray, _trace: bool = False):
    global LAST_RESULTS, _CURRENT_NC
    from concourse.bass_utils import run_bass_kernel_spmd

    I = np.asarray(I, dtype=np.float32)
    raw_tau = np.asarray(raw_tau, dtype=np.float32)
    assert I.shape == (B, F, L), I.shape

    alpha, om = _alpha_host(raw_tau)
    theta = (np.float32(THR) / om).astype(np.float32)
    sc_z = (np.float32(BETA) * om).astype(np.float32)
    sg_b = (-np.float32(SIGM) * theta).astype(np.float32)

    nc = _get_nc()
    _CURRENT_NC = nc

    in_maps = []
    for c in range(N_CORES):
        fb, q = c % 2, c // 2
        fsl = slice(fb * FB, (fb + 1) * FB)
        IT = np.ascontiguousarray(I[:, fsl, :].transpose(1, 2, 0))  # [128, L, 64]
        iD, iP = _pack_core(IT, q)
        cst = np.stack(
            [alpha[fsl], theta[fsl], om[fsl], sc_z[fsl], sg_b[fsl]], axis=1
        ).astype(np.float32)
        in_maps.append(
            {"i_dve": iD, "i_pool": iP, "consts": np.ascontiguousarray(cst)}
        )

    res = run_bass_kernel_spmd(nc, in_maps, core_ids=list(range(N_CORES)), trace=_trace)
    LAST_RESULTS = res

    v = np.empty((B, F, L), np.float32)
    z = np.empty((B, F, L), np.float32)
    s = np.empty((B, F, L), np.float32)
    for c in range(N_CORES):
        fb, q = c % 2, c // 2
        fsl = slice(fb * FB, (fb + 1) * FB)
        r0 = q * QLEN
        r = res.results[c]
        nod = TD // TC
        for name, dst in (("v_dve", v), ("z_dve", z), ("s_dve", s)):
            a = np.asarray(r[name]).astype(np.float32).reshape(128, nod, 2, TC, KD, HB)
            a = a.transpose(0, 1, 3, 4, 2, 5).reshape(128, TD, KD, 64)
            for j in range(KD):
                t0 = r0 + j * TD
                dst[:, fsl, t0 : t0 + TD] = a[:, :, j, :].transpose(2, 0, 1)
        nop = TP // TC
        for name, dst in (("v_pool", v), ("z_pool", z), ("s_pool", s)):
            a = np.asarray(r[name]).astype(np.float32).reshape(128, nop, 2, TC, HB)
            a = a.transpose(0, 1, 3, 2, 4).reshape(128, TP, 64)
            t0 = r0 + KD * TD
            dst[:, fsl, t0 : t0 + TP] = a.transpose(2, 0, 1)
    return v, z, s


# revision 25
# speedup vs baseline: 1.0459x; 1.0459x over previous
"""LIF layer (leaky integrate-and-fire scan over time) on 8 Trainium2 cores.

Recurrence per (b, f) row over t = 0..L-1 (reference semantics):
    v_pre[t] = alpha[f] * v[t-1] + (1 - alpha[f]) * I[b, f, t]
    z[t]     = BETA * (v_pre[t] - THR)
    s[t]     = (v_pre[t] >= THR)
    v[t]     = v_pre[t] * (v_pre[t] < THR)          # reset on spike

Outputs: (v_pre, z, s) each [B, F, L] float32.

Device algorithm:
- Scaled state u = v / (1 - alpha) turns the step into u' = alpha*u + I[t]
  (raw input feeds the chain directly, no J=(1-alpha)*I precompute) with the
  reset compare against a per-partition threshold theta = THR/(1-alpha):
      STT1: u_pre = (u * alpha) + I[t]
      STT2: u     = (u_pre < theta) * u_pre
- Sharding: 8 cores = 2 f-halves x 4 time-quarters of 512 steps. Within a
  quarter the serial scan is split across engines and segments:
    DVE:  4 segments of 112 steps, fused into one op pair (free = 4 segs).
    Pool: 1 segment of 64 steps (no scalar_tensor_tensor in the Pool ISA;
          uses the 3-op form m=(u<theta)*alpha; g=u*m; u'=g+I).
  Each segment re-converges state with a 32-step warmup (decay + spike-reset
  healing put the state within fp32 noise of the true trajectory).
- Latency hiding: each engine runs TWO interleaved chain groups over the two
  batch halves (b 0:32 / 32:64). Consecutive ops of one group are 2 issue
  slots apart, which covers the same-engine semaphore dependency latency, so
  the engines run at their raw per-op rate.
- Outputs on ScalarE from the f32 u_pre chunks:
      v = bf16(omalpha * u_pre)                    (Copy, per-partition scale)
      z = bf16(15*omalpha * u_pre - 3.75)          (Copy, scale + const bias)
      s = fp8(sigmoid(1e30 * u_pre - 1e30*theta))  (saturates to exact 0/1)
  Host upcasts to f32; norm rel-err from bf16 is ~1.7e-3, inside 2e-2.
"""

import sys

sys.path.insert(0, "/opt/trn_rl_repo")

import numpy as np

DT = 1.0
BETA = 15.0
THR = 0.25

B, F, L = 64, 256, 2048
N_CORES = 8
FB = 128          # partition block of F per core
NQ = 4            # time quarters
QLEN = L // NQ    # 512
TC = 16           # time-chunk length
W = 32            # warmup steps per segment
KD = 4            # DVE fused segment count
TD = 112          # DVE segment length
TP = 64           # Pool segment length
HB = 32           # half-batch for the two interleaved chain groups
SIGM = 1.0e30     # sigmoid saturation scale for the spike output

NDC = (W + TD) // TC   # 12 DVE chunks (3 warmup + 9 output)
NPC = (W + TP) // TC   # 8 Pool chunks (3 warmup + 5 output)
NWC = W // TC          # 3

_BUILD_CACHE: dict = {}
LAST_RESULTS = None  # BassKernelResults of the most recent kernel() call
_CURRENT_NC = None


def _get_current_nc():
    return _CURRENT_NC


def _build_v5():
    import concourse.bacc as bacc
    import concourse.mybir as mybir
    from concourse import tile

    f32 = mybir.dt.float32
    bf16 = mybir.dt.bfloat16
    fp8 = mybir.dt.float8e4
    Alu = mybir.AluOpType
    Act = mybir.ActivationFunctionType

    nc = bacc.Bacc(None, target_bir_lowering=False)

    iD = nc.dram_tensor("i_dve", [128, NDC, 2, TC, KD, HB], f32, kind="ExternalInput")
    iP = nc.dram_tensor("i_pool", [128, NPC, 2, TC, HB], f32, kind="ExternalInput")
    cst_d = nc.dram_tensor("consts", [128, 5], f32, kind="ExternalInput")

    nod, nop = NDC - NWC, NPC - NWC
    vD = nc.dram_tensor("v_dve", [128, nod, 2, TC, KD, HB], bf16, kind="ExternalOutput")
    zD = nc.dram_tensor("z_dve", [128, nod, 2, TC, KD, HB], bf16, kind="ExternalOutput")
    sD = nc.dram_tensor("s_dve", [128, nod, 2, TC, KD, HB], fp8, kind="ExternalOutput")
    vP = nc.dram_tensor("v_pool", [128, nop, 2, TC, HB], bf16, kind="ExternalOutput")
    zP = nc.dram_tensor("z_pool", [128, nop, 2, TC, HB], bf16, kind="ExternalOutput")
    sP = nc.dram_tensor("s_pool", [128, nop, 2, TC, HB], fp8, kind="ExternalOutput")

    with tile.TileContext(nc) as tc_:
        with (
            tc_.tile_pool(name="const", bufs=1) as cp,
            tc_.tile_pool(name="din", bufs=4) as din,
            tc_.tile_pool(name="dwork", bufs=3) as dw,
            tc_.tile_pool(name="dout", bufs=2) as do_,
            tc_.tile_pool(name="pin", bufs=5) as pin,
            tc_.tile_pool(name="pwork", bufs=3) as pw,
            tc_.tile_pool(name="pout", bufs=2) as po,
        ):
            # first input chunks, split fine so the chains can start early;
            # consts ride the ACT ring in one packed transfer.
            it0 = din.tile([128, 2, TC, KD, HB], f32, tag="di")
            for h in (0, 1):
                nc.sync.dma_start(it0[:, h, 0 : TC // 2], iD[:, 0, h, 0 : TC // 2])
            ip0 = pin.tile([128, 2, TC, HB], f32, tag="pi")
            for h in (0, 1):
                nc.sync.dma_start(ip0[:, h, 0 : TC // 2], iP[:, 0, h, 0 : TC // 2])
            for h in (0, 1):
                nc.sync.dma_start(it0[:, h, TC // 2 :], iD[:, 0, h, TC // 2 :])
                nc.sync.dma_start(ip0[:, h, TC // 2 :], iP[:, 0, h, TC // 2 :])

            cst_t = cp.tile([128, 5], f32, tag="cst")
            nc.scalar.dma_start(cst_t[:], cst_d[:])
            al_t = cst_t[:, 0:1]
            th_t = cst_t[:, 1:2]
            sv_t = cst_t[:, 2:3]
            sz_t = cst_t[:, 3:4]
            sb_t = cst_t[:, 4:5]

            ustD = cp.tile([128, 2, KD, HB], f32, tag="ustD")
            upwD = cp.tile([128, 2, KD, HB], f32, tag="upwD")
            nc.gpsimd.memset(ustD[:], 0.0)
            # Pool state holds u_pre (pre-reset); m/g are per-half scratch.
            upwP = cp.tile([128, 2, HB], f32, tag="upwP")
            mP = cp.tile([128, 2, HB], f32, tag="mP")
            gP = cp.tile([128, 2, HB], f32, tag="gP")
            nc.gpsimd.memset(upwP[:], 0.0)
            uprevP = [upwP[:, 0], upwP[:, 1]]
            pending_pout = []

            for k in range(NDC):
                # ---- DVE: KD fused segments, 2 interleaved b-half groups ----
                if k == 0:
                    it = it0
                else:
                    it = din.tile([128, 2, TC, KD, HB], f32, tag="di")
                    nc.sync.dma_start(it[:], iD[:, k])
                is_out = k >= NWC
                up = None
                if is_out:
                    up = dw.tile([128, 2, TC, KD, HB], f32, tag="dup")
                for t in range(TC):
                    if is_out:
                        dsts = (up[:, 0, t], up[:, 1, t])
                    else:
                        dsts = (upwD[:, 0], upwD[:, 1])
                    for h in (0, 1):
                        nc.vector.scalar_tensor_tensor(
                            dsts[h], ustD[:, h], al_t, it[:, h, t],
                            op0=Alu.mult, op1=Alu.add,
                        )
                    for h in (0, 1):
                        nc.vector.scalar_tensor_tensor(
                            ustD[:, h], dsts[h], th_t, dsts[h],
                            op0=Alu.is_lt, op1=Alu.mult,
                        )
                # deferred pool-chunk v/s on DVE (pool chunk k-1 is done)
                if pending_pout:
                    po_, pupp = pending_pout.pop(0)
                    pvt = po.tile([128, 2, TC, HB], bf16, tag="pv")
                    nc.vector.tensor_scalar(pvt[:], pupp[:], sv_t, None, Alu.mult)
                    pst = po.tile([128, 2, TC, HB], fp8, tag="ps")
                    nc.vector.tensor_scalar(pst[:], pupp[:], th_t, None, Alu.is_ge)
                    pzt = po.tile([128, 2, TC, HB], bf16, tag="pz")
                    nc.scalar.activation(pzt[:], pupp[:], Act.Copy, bias=-BETA * THR, scale=sz_t)
                    nc.scalar.dma_start(vP[:, po_], pvt[:])
                    nc.scalar.dma_start(zP[:, po_], pzt[:])
                    nc.scalar.dma_start(sP[:, po_], pst[:])

                if is_out:
                    o = k - NWC
                    vt = do_.tile([128, 2, TC, KD, HB], bf16, tag="dv")
                    zt = do_.tile([128, 2, TC, KD, HB], bf16, tag="dz")
                    st = do_.tile([128, 2, TC, KD, HB], fp8, tag="ds")
                    if k == NDC - 1 or k == NWC:
                        # tail chunk (chain engines free) and first output
                        # chunk (ACT has no release credits yet): v/s on DVE
                        nc.vector.tensor_scalar(vt[:], up[:], sv_t, None, Alu.mult)
                        nc.vector.tensor_scalar(st[:], up[:], th_t, None, Alu.is_ge)
                    elif k >= NPC - 1:
                        # Pool chain is done by now; use it for v/s
                        nc.gpsimd.tensor_scalar(vt[:], up[:], sv_t, None, Alu.mult)
                        nc.gpsimd.tensor_scalar(st[:], up[:], th_t, None, Alu.is_ge)
                    else:
                        nc.scalar.activation(vt[:], up[:], Act.Copy, bias=0.0, scale=sv_t)
                        nc.scalar.activation(st[:], up[:], Act.Sigmoid, bias=sb_t, scale=SIGM)
                    nc.scalar.activation(zt[:], up[:], Act.Copy, bias=-BETA * THR, scale=sz_t)
                    nc.scalar.dma_start(vD[:, o], vt[:])
                    nc.scalar.dma_start(zD[:, o], zt[:])
                    nc.scalar.dma_start(sD[:, o], st[:])

                # ---- Pool: 1 segment, 3-op form, 2 interleaved b-half groups ----
                if k >= NPC:
                    continue
                if k == 0:
                    ip = ip0
                else:
                    ip = pin.tile([128, 2, TC, HB], f32, tag="pi")
                    nc.sync.dma_start(ip[:], iP[:, k])
                is_out = k >= NWC
                upp = None
                if is_out:
                    upp = pw.tile([128, 2, TC, HB], f32, tag="pup")
                for t in range(TC):
                    if is_out:
                        dsts = (upp[:, 0, t], upp[:, 1, t])
                    else:
                        dsts = (upwP[:, 0], upwP[:, 1])
                    for h in (0, 1):
                        nc.gpsimd.tensor_scalar(
                            mP[:, h], uprevP[h], th_t, al_t,
                            Alu.is_lt, Alu.mult,
                        )
                    for h in (0, 1):
                        nc.gpsimd.tensor_mul(gP[:, h], uprevP[h], mP[:, h])
                    for h in (0, 1):
                        nc.gpsimd.tensor_add(dsts[h], gP[:, h], ip[:, h, t])
                    uprevP = [dsts[0], dsts[1]]
                if is_out:
                    pending_pout.append((k - NWC, upp))

            # flush remaining pool outputs (after all chains)
            for po_, pupp in pending_pout:
                pvt = po.tile([128, 2, TC, HB], bf16, tag="pv")
                nc.vector.tensor_scalar(pvt[:], pupp[:], sv_t, None, Alu.mult)
                pst = po.tile([128, 2, TC, HB], fp8, tag="ps")
                nc.vector.tensor_scalar(pst[:], pupp[:], th_t, None, Alu.is_ge)
                pzt = po.tile([128, 2, TC, HB], bf16, tag="pz")
                nc.scalar.activation(pzt[:], pupp[:], Act.Copy, bias=-BETA * THR, scale=sz_t)
                nc.scalar.dma_start(vP[:, po_], pvt[:])
                nc.scalar.dma_start(zP[:, po_], pzt[:])
                nc.scalar.dma_start(sP[:, po_], pst[:])

    nc.compile()
    return nc


def _get_nc():
    key = ("v5", TC, W, KD, TD, TP)
    if key not in _BUILD_CACHE:
        _BUILD_CACHE[key] = _build_v5()
    return _BUILD_CACHE[key]


# --------------------------------------------------------------------------
# Host side


def _alpha_host(raw_tau: np.ndarray) -> tuple[np.ndarray, np.ndarray]:
    """alpha = exp(-DT / (softplus(raw_tau) + 1e-4)) with the same jax ops /
    device as the reference, so spike threshold comparisons match closely."""
    import jax
    import jax.numpy as jnp

    with jax.default_device(jax.devices("cpu")[0]):
        tau = jax.nn.softplus(jnp.asarray(np.asarray(raw_tau))) + 1e-4
        alpha = np.asarray(jnp.exp(-DT / tau), dtype=np.float32)
    one_minus = (np.float32(1.0) - alpha).astype(np.float32)
    return alpha, one_minus


def _pack_core(IT: np.ndarray, q: int) -> tuple[np.ndarray, np.ndarray]:
    """IT: [128, L, 64] (f, t, b) for this core's f-block. Returns the DVE
    and Pool input streams with warmup prefixes (zero-padded below t=0)."""
    r0 = q * QLEN
    tD = np.empty((W + TD, KD), np.int64)
    for j in range(KD):
        start = r0 + j * TD
        tD[:, j] = np.arange(start - W, start + TD)
    tP = np.arange(r0 + KD * TD - W, r0 + QLEN)

    mD = tD >= 0
    mP = tP >= 0
    iD = IT[:, np.clip(tD, 0, L - 1), :]        # [128, W+TD, KD, 64]
    iD[:, ~mD] = 0.0
    iP = IT[:, np.clip(tP, 0, L - 1), :]        # [128, W+TP, 64]
    iP[:, ~mP] = 0.0
    # device layout: [128, chunk, h(2), t(TC), KD, HB] with b = h*HB + b'
    iD = iD.reshape(128, NDC, TC, KD, 2, HB).transpose(0, 1, 4, 2, 3, 5)
    iP = iP.reshape(128, NPC, TC, 2, HB).transpose(0, 1, 3, 2, 4)
    return np.ascontiguousarray(iD), np.ascontiguousarray(iP)


def kernel(I: np.ndarray, raw_tau: np.ndarray, _trace: bool = False):
    global LAST_RESULTS, _CURRENT_NC
    from concourse.bass_utils import run_bass_kernel_spmd

    I = np.asarray(I, dtype=np.float32)
    raw_tau = np.asarray(raw_tau, dtype=np.float32)
    assert I.shape == (B, F, L), I.shape

    alpha, om = _alpha_host(raw_tau)
    theta = (np.float32(THR) / om).astype(np.float32)
    sc_z = (np.float32(BETA) * om).astype(np.float32)
    sg_b = (-np.float32(SIGM) * theta).astype(np.float32)

    nc = _get_nc()
    _CURRENT_NC = nc

    in_maps = []
    for c in range(N_CORES):
        fb, q = c % 2, c // 2
        fsl = slice(fb * FB, (fb + 1) * FB)
        IT = np.ascontiguousarray(I[:, fsl, :].transpose(1, 2, 0))  # [128, L, 64]
        iD, iP = _pack_core(IT, q)
        cst = np.stack(
            [alpha[fsl], theta[fsl], om[fsl], sc_z[fsl], sg_b[fsl]], axis=1
        ).astype(np.float32)
        in_maps.append(
            {"i_dve": iD, "i_pool": iP, "consts": np.ascontiguousarray(cst)}
        )

    res = run_bass_kernel_spmd(nc, in_maps, core_ids=list(range(N_CORES)), trace=_trace)
    LAST_RESULTS = res

    v = np.empty((B, F, L), np.float32)
    z = np.empty((B, F, L), np.float32)
    s = np.empty((B, F, L), np.float32)
    for c in range(N_CORES):
        fb, q = c % 2, c // 2
        fsl = slice(fb * FB, (fb + 1) * FB)
        r0 = q * QLEN
        r = res.results[c]
        nod = TD // TC
        for name, dst in (("v_dve", v), ("z_dve", z), ("s_dve", s)):
            a = np.asarray(r[name]).astype(np.float32).reshape(128, nod, 2, TC, KD, HB)
            a = a.transpose(0, 1, 3, 4, 2, 5).reshape(128, TD, KD, 64)
            for j in range(KD):
                t0 = r0 + j * TD
                dst[:, fsl, t0 : t0 + TD] = a[:, :, j, :].transpose(2, 0, 1)
        nop = TP // TC
        for name, dst in (("v_pool", v), ("z_pool", z), ("s_pool", s)):
            a = np.asarray(r[name]).astype(np.float32).reshape(128, nop, 2, TC, HB)
            a = a.transpose(0, 1, 3, 2, 4).reshape(128, TP, 64)
            t0 = r0 + KD * TD
            dst[:, fsl, t0 : t0 + TP] = a.transpose(2, 0, 1)
    return v, z, s


# revision 26
# speedup vs baseline: 1.1597x; 1.1088x over previous
"""LIF layer (leaky integrate-and-fire scan over time) on 8 Trainium2 cores.

Recurrence per (b, f) row over t = 0..L-1 (reference semantics):
    v_pre[t] = alpha[f] * v[t-1] + (1 - alpha[f]) * I[b, f, t]
    z[t]     = BETA * (v_pre[t] - THR)
    s[t]     = (v_pre[t] >= THR)
    v[t]     = v_pre[t] * (v_pre[t] < THR)          # reset on spike

Outputs: (v_pre, z, s) each [B, F, L] float32.

Device algorithm:
- Scaled state u = v / (1 - alpha) turns the step into u' = alpha*u + I[t]
  (raw input feeds the chain directly, no J=(1-alpha)*I precompute) with the
  reset compare against a per-partition threshold theta = THR/(1-alpha):
      STT1: u_pre = (u * alpha) + I[t]
      STT2: u     = (u_pre < theta) * u_pre
- Sharding: 8 cores = 2 f-halves x 4 time-quarters of 512 steps. Within a
  quarter the serial scan is split across engines and segments:
    DVE:  4 segments of 112 steps, fused into one op pair (free = 4 segs).
    Pool: 1 segment of 64 steps (no scalar_tensor_tensor in the Pool ISA;
          uses the 3-op form m=(u<theta)*alpha; g=u*m; u'=g+I).
  Each segment re-converges state with a 32-step warmup (decay + spike-reset
  healing put the state within fp32 noise of the true trajectory).
- Latency hiding: each engine runs TWO interleaved chain groups over the two
  batch halves (b 0:32 / 32:64). Consecutive ops of one group are 2 issue
  slots apart, which covers the same-engine semaphore dependency latency, so
  the engines run at their raw per-op rate.
- Outputs on ScalarE from the f32 u_pre chunks:
      v = bf16(omalpha * u_pre)                    (Copy, per-partition scale)
      z = bf16(15*omalpha * u_pre - 3.75)          (Copy, scale + const bias)
      s = fp8(sigmoid(1e30 * u_pre - 1e30*theta))  (saturates to exact 0/1)
  Host upcasts to f32; norm rel-err from bf16 is ~1.7e-3, inside 2e-2.
"""

import sys

sys.path.insert(0, "/opt/trn_rl_repo")

import numpy as np

DT = 1.0
BETA = 15.0
THR = 0.25

B, F, L = 64, 256, 2048
N_CORES = 8
FB = 128          # partition block of F per core
NQ = 4            # time quarters
QLEN = L // NQ    # 512
TC = 16           # time-chunk length
W = 16            # warmup steps per segment
KD = 4            # DVE fused segment count
TD = 112          # DVE segment length
TP = 64           # Pool segment length
HB = 32           # half-batch for the two interleaved chain groups
SIGM = 1.0e30     # sigmoid saturation scale for the spike output

NDC = (W + TD) // TC   # 12 DVE chunks (3 warmup + 9 output)
NPC = (W + TP) // TC   # 8 Pool chunks (3 warmup + 5 output)
NWC = W // TC          # 3

_BUILD_CACHE: dict = {}
LAST_RESULTS = None  # BassKernelResults of the most recent kernel() call
_CURRENT_NC = None


def _get_current_nc():
    return _CURRENT_NC


def _build_v5():
    import concourse.bacc as bacc
    import concourse.mybir as mybir
    from concourse import tile

    f32 = mybir.dt.float32
    bf16 = mybir.dt.bfloat16
    fp8 = mybir.dt.float8e4
    Alu = mybir.AluOpType
    Act = mybir.ActivationFunctionType

    nc = bacc.Bacc(None, target_bir_lowering=False)

    iD = nc.dram_tensor("i_dve", [128, NDC, 2, TC, KD, HB], f32, kind="ExternalInput")
    iP = nc.dram_tensor("i_pool", [128, NPC, 2, TC, HB], f32, kind="ExternalInput")
    cst_d = nc.dram_tensor("consts", [128, 5], f32, kind="ExternalInput")

    nod, nop = NDC - NWC, NPC - NWC
    vD = nc.dram_tensor("v_dve", [128, nod, 2, TC, KD, HB], bf16, kind="ExternalOutput")
    zD = nc.dram_tensor("z_dve", [128, nod, 2, TC, KD, HB], bf16, kind="ExternalOutput")
    sD = nc.dram_tensor("s_dve", [128, nod, 2, TC, KD, HB], fp8, kind="ExternalOutput")
    vP = nc.dram_tensor("v_pool", [128, nop, 2, TC, HB], bf16, kind="ExternalOutput")
    zP = nc.dram_tensor("z_pool", [128, nop, 2, TC, HB], bf16, kind="ExternalOutput")
    sP = nc.dram_tensor("s_pool", [128, nop, 2, TC, HB], fp8, kind="ExternalOutput")

    with tile.TileContext(nc) as tc_:
        with (
            tc_.tile_pool(name="const", bufs=1) as cp,
            tc_.tile_pool(name="din", bufs=4) as din,
            tc_.tile_pool(name="dwork", bufs=3) as dw,
            tc_.tile_pool(name="dout", bufs=2) as do_,
            tc_.tile_pool(name="pin", bufs=5) as pin,
            tc_.tile_pool(name="pwork", bufs=3) as pw,
            tc_.tile_pool(name="pout", bufs=2) as po,
        ):
            # first input chunks, split fine so the chains can start early;
            # consts ride the ACT ring in one packed transfer.
            it0 = din.tile([128, 2, TC, KD, HB], f32, tag="di")
            for h in (0, 1):
                nc.sync.dma_start(it0[:, h, 0 : TC // 2], iD[:, 0, h, 0 : TC // 2])
            ip0 = pin.tile([128, 2, TC, HB], f32, tag="pi")
            for h in (0, 1):
                nc.sync.dma_start(ip0[:, h, 0 : TC // 2], iP[:, 0, h, 0 : TC // 2])
            for h in (0, 1):
                nc.sync.dma_start(it0[:, h, TC // 2 :], iD[:, 0, h, TC // 2 :])
                nc.sync.dma_start(ip0[:, h, TC // 2 :], iP[:, 0, h, TC // 2 :])

            cst_t = cp.tile([128, 5], f32, tag="cst")
            nc.scalar.dma_start(cst_t[:], cst_d[:])
            al_t = cst_t[:, 0:1]
            th_t = cst_t[:, 1:2]
            sv_t = cst_t[:, 2:3]
            sz_t = cst_t[:, 3:4]
            sb_t = cst_t[:, 4:5]

            ustD = cp.tile([128, 2, KD, HB], f32, tag="ustD")
            upwD = cp.tile([128, 2, KD, HB], f32, tag="upwD")
            nc.gpsimd.memset(ustD[:], 0.0)
            # Pool state holds u_pre (pre-reset); m/g are per-half scratch.
            upwP = cp.tile([128, 2, HB], f32, tag="upwP")
            mP = cp.tile([128, 2, HB], f32, tag="mP")
            gP = cp.tile([128, 2, HB], f32, tag="gP")
            nc.gpsimd.memset(upwP[:], 0.0)
            uprevP = [upwP[:, 0], upwP[:, 1]]
            pending_pout = []

            for k in range(NDC):
                # ---- DVE: KD fused segments, 2 interleaved b-half groups ----
                if k == 0:
                    it = it0
                else:
                    it = din.tile([128, 2, TC, KD, HB], f32, tag="di")
                    nc.sync.dma_start(it[:], iD[:, k])
                is_out = k >= NWC
                up = None
                if is_out:
                    up = dw.tile([128, 2, TC, KD, HB], f32, tag="dup")
                for t in range(TC):
                    if is_out:
                        dsts = (up[:, 0, t], up[:, 1, t])
                    else:
                        dsts = (upwD[:, 0], upwD[:, 1])
                    for h in (0, 1):
                        nc.vector.scalar_tensor_tensor(
                            dsts[h], ustD[:, h], al_t, it[:, h, t],
                            op0=Alu.mult, op1=Alu.add,
                        )
                    for h in (0, 1):
                        nc.vector.scalar_tensor_tensor(
                            ustD[:, h], dsts[h], th_t, dsts[h],
                            op0=Alu.is_lt, op1=Alu.mult,
                        )
                # deferred pool-chunk v/s on DVE (pool chunk k-1 is done)
                if pending_pout:
                    po_, pupp = pending_pout.pop(0)
                    pvt = po.tile([128, 2, TC, HB], bf16, tag="pv")
                    nc.vector.tensor_scalar(pvt[:], pupp[:], sv_t, None, Alu.mult)
                    pst = po.tile([128, 2, TC, HB], fp8, tag="ps")
                    nc.vector.tensor_scalar(pst[:], pupp[:], th_t, None, Alu.is_ge)
                    pzt = po.tile([128, 2, TC, HB], bf16, tag="pz")
                    nc.scalar.activation(pzt[:], pupp[:], Act.Copy, bias=-BETA * THR, scale=sz_t)
                    nc.scalar.dma_start(vP[:, po_], pvt[:])
                    nc.scalar.dma_start(zP[:, po_], pzt[:])
                    nc.scalar.dma_start(sP[:, po_], pst[:])

                if is_out:
                    o = k - NWC
                    vt = do_.tile([128, 2, TC, KD, HB], bf16, tag="dv")
                    zt = do_.tile([128, 2, TC, KD, HB], bf16, tag="dz")
                    st = do_.tile([128, 2, TC, KD, HB], fp8, tag="ds")
                    if k == NDC - 1 or k == NWC:
                        # tail chunk (chain engines free) and first output
                        # chunk (ACT has no release credits yet): v/s on DVE
                        nc.vector.tensor_scalar(vt[:], up[:], sv_t, None, Alu.mult)
                        nc.vector.tensor_scalar(st[:], up[:], th_t, None, Alu.is_ge)
                    elif k >= NPC - 1:
                        # Pool chain is done by now; use it for v/s
                        nc.gpsimd.tensor_scalar(vt[:], up[:], sv_t, None, Alu.mult)
                        nc.gpsimd.tensor_scalar(st[:], up[:], th_t, None, Alu.is_ge)
                    else:
                        nc.scalar.activation(vt[:], up[:], Act.Copy, bias=0.0, scale=sv_t)
                        nc.scalar.activation(st[:], up[:], Act.Sigmoid, bias=sb_t, scale=SIGM)
                    nc.scalar.activation(zt[:], up[:], Act.Copy, bias=-BETA * THR, scale=sz_t)
                    nc.scalar.dma_start(vD[:, o], vt[:])
                    nc.scalar.dma_start(zD[:, o], zt[:])
                    nc.scalar.dma_start(sD[:, o], st[:])

                # ---- Pool: 1 segment, 3-op form, 2 interleaved b-half groups ----
                if k >= NPC:
                    continue
                if k == 0:
                    ip = ip0
                else:
                    ip = pin.tile([128, 2, TC, HB], f32, tag="pi")
                    nc.sync.dma_start(ip[:], iP[:, k])
                is_out = k >= NWC
                upp = None
                if is_out:
                    upp = pw.tile([128, 2, TC, HB], f32, tag="pup")
                for t in range(TC):
                    if is_out:
                        dsts = (upp[:, 0, t], upp[:, 1, t])
                    else:
                        dsts = (upwP[:, 0], upwP[:, 1])
                    for h in (0, 1):
                        nc.gpsimd.tensor_scalar(
                            mP[:, h], uprevP[h], th_t, al_t,
                            Alu.is_lt, Alu.mult,
                        )
                    for h in (0, 1):
                        nc.gpsimd.tensor_mul(gP[:, h], uprevP[h], mP[:, h])
                    for h in (0, 1):
                        nc.gpsimd.tensor_add(dsts[h], gP[:, h], ip[:, h, t])
                    uprevP = [dsts[0], dsts[1]]
                if is_out:
                    pending_pout.append((k - NWC, upp))

            # flush remaining pool outputs (after all chains)
            for po_, pupp in pending_pout:
                pvt = po.tile([128, 2, TC, HB], bf16, tag="pv")
                nc.vector.tensor_scalar(pvt[:], pupp[:], sv_t, None, Alu.mult)
                pst = po.tile([128, 2, TC, HB], fp8, tag="ps")
                nc.vector.tensor_scalar(pst[:], pupp[:], th_t, None, Alu.is_ge)
                pzt = po.tile([128, 2, TC, HB], bf16, tag="pz")
                nc.scalar.activation(pzt[:], pupp[:], Act.Copy, bias=-BETA * THR, scale=sz_t)
                nc.scalar.dma_start(vP[:, po_], pvt[:])
                nc.scalar.dma_start(zP[:, po_], pzt[:])
                nc.scalar.dma_start(sP[:, po_], pst[:])

    nc.compile()
    return nc


def _get_nc():
    key = ("v5", TC, W, KD, TD, TP)
    if key not in _BUILD_CACHE:
        _BUILD_CACHE[key] = _build_v5()
    return _BUILD_CACHE[key]


# --------------------------------------------------------------------------
# Host side


def _alpha_host(raw_tau: np.ndarray) -> tuple[np.ndarray, np.ndarray]:
    """alpha = exp(-DT / (softplus(raw_tau) + 1e-4)) with the same jax ops /
    device as the reference, so spike threshold comparisons match closely."""
    import jax
    import jax.numpy as jnp

    with jax.default_device(jax.devices("cpu")[0]):
        tau = jax.nn.softplus(jnp.asarray(np.asarray(raw_tau))) + 1e-4
        alpha = np.asarray(jnp.exp(-DT / tau), dtype=np.float32)
    one_minus = (np.float32(1.0) - alpha).astype(np.float32)
    return alpha, one_minus


def _pack_core(IT: np.ndarray, q: int) -> tuple[np.ndarray, np.ndarray]:
    """IT: [128, L, 64] (f, t, b) for this core's f-block. Returns the DVE
    and Pool input streams with warmup prefixes (zero-padded below t=0)."""
    r0 = q * QLEN
    tD = np.empty((W + TD, KD), np.int64)
    for j in range(KD):
        start = r0 + j * TD
        tD[:, j] = np.arange(start - W, start + TD)
    tP = np.arange(r0 + KD * TD - W, r0 + QLEN)

    mD = tD >= 0
    mP = tP >= 0
    iD = IT[:, np.clip(tD, 0, L - 1), :]        # [128, W+TD, KD, 64]
    iD[:, ~mD] = 0.0
    iP = IT[:, np.clip(tP, 0, L - 1), :]        # [128, W+TP, 64]
    iP[:, ~mP] = 0.0
    # device layout: [128, chunk, h(2), t(TC), KD, HB] with b = h*HB + b'
    iD = iD.reshape(128, NDC, TC, KD, 2, HB).transpose(0, 1, 4, 2, 3, 5)
    iP = iP.reshape(128, NPC, TC, 2, HB).transpose(0, 1, 3, 2, 4)
    return np.ascontiguousarray(iD), np.ascontiguousarray(iP)


def kernel(I: np.ndarray, raw_tau: np.ndarray, _trace: bool = False):
    global LAST_RESULTS, _CURRENT_NC
    from concourse.bass_utils import run_bass_kernel_spmd

    I = np.asarray(I, dtype=np.float32)
    raw_tau = np.asarray(raw_tau, dtype=np.float32)
    assert I.shape == (B, F, L), I.shape

    alpha, om = _alpha_host(raw_tau)
    theta = (np.float32(THR) / om).astype(np.float32)
    sc_z = (np.float32(BETA) * om).astype(np.float32)
    sg_b = (-np.float32(SIGM) * theta).astype(np.float32)

    nc = _get_nc()
    _CURRENT_NC = nc

    in_maps = []
    for c in range(N_CORES):
        fb, q = c % 2, c // 2
        fsl = slice(fb * FB, (fb + 1) * FB)
        IT = np.ascontiguousarray(I[:, fsl, :].transpose(1, 2, 0))  # [128, L, 64]
        iD, iP = _pack_core(IT, q)
        cst = np.stack(
            [alpha[fsl], theta[fsl], om[fsl], sc_z[fsl], sg_b[fsl]], axis=1
        ).astype(np.float32)
        in_maps.append(
            {"i_dve": iD, "i_pool": iP, "consts": np.ascontiguousarray(cst)}
        )

    res = run_bass_kernel_spmd(nc, in_maps, core_ids=list(range(N_CORES)), trace=_trace)
    LAST_RESULTS = res

    v = np.empty((B, F, L), np.float32)
    z = np.empty((B, F, L), np.float32)
    s = np.empty((B, F, L), np.float32)
    for c in range(N_CORES):
        fb, q = c % 2, c // 2
        fsl = slice(fb * FB, (fb + 1) * FB)
        r0 = q * QLEN
        r = res.results[c]
        nod = TD // TC
        for name, dst in (("v_dve", v), ("z_dve", z), ("s_dve", s)):
            a = np.asarray(r[name]).astype(np.float32).reshape(128, nod, 2, TC, KD, HB)
            a = a.transpose(0, 1, 3, 4, 2, 5).reshape(128, TD, KD, 64)
            for j in range(KD):
                t0 = r0 + j * TD
                dst[:, fsl, t0 : t0 + TD] = a[:, :, j, :].transpose(2, 0, 1)
        nop = TP // TC
        for name, dst in (("v_pool", v), ("z_pool", z), ("s_pool", s)):
            a = np.asarray(r[name]).astype(np.float32).reshape(128, nop, 2, TC, HB)
            a = a.transpose(0, 1, 3, 2, 4).reshape(128, TP, 64)
            t0 = r0 + KD * TD
            dst[:, fsl, t0 : t0 + TP] = a.transpose(2, 0, 1)
    return v, z, s


# revision 30
# speedup vs baseline: 1.1610x; 1.0011x over previous
"""LIF layer (leaky integrate-and-fire scan over time) on 8 Trainium2 cores.

Recurrence per (b, f) row over t = 0..L-1 (reference semantics):
    v_pre[t] = alpha[f] * v[t-1] + (1 - alpha[f]) * I[b, f, t]
    z[t]     = BETA * (v_pre[t] - THR)
    s[t]     = (v_pre[t] >= THR)
    v[t]     = v_pre[t] * (v_pre[t] < THR)          # reset on spike

Outputs: (v_pre, z, s) each [B, F, L] float32.

Device algorithm:
- Scaled state u = v / (1 - alpha) turns the step into u' = alpha*u + I[t]
  (raw input feeds the chain directly, no J=(1-alpha)*I precompute) with the
  reset compare against a per-partition threshold theta = THR/(1-alpha):
      STT1: u_pre = (u * alpha) + I[t]
      STT2: u     = (u_pre < theta) * u_pre
- Sharding: 8 cores = 2 f-halves x 4 time-quarters of 512 steps. Within a
  quarter the serial scan is split across engines and segments:
    DVE:  4 segments of 112 steps, fused into one op pair (free = 4 segs).
    Pool: 1 segment of 64 steps (no scalar_tensor_tensor in the Pool ISA;
          uses the 3-op form m=(u<theta)*alpha; g=u*m; u'=g+I).
  Each segment re-converges state with a 32-step warmup (decay + spike-reset
  healing put the state within fp32 noise of the true trajectory).
- Latency hiding: each engine runs TWO interleaved chain groups over the two
  batch halves (b 0:32 / 32:64). Consecutive ops of one group are 2 issue
  slots apart, which covers the same-engine semaphore dependency latency, so
  the engines run at their raw per-op rate.
- Outputs on ScalarE from the f32 u_pre chunks:
      v = bf16(omalpha * u_pre)                    (Copy, per-partition scale)
      z = bf16(15*omalpha * u_pre - 3.75)          (Copy, scale + const bias)
      s = fp8(sigmoid(1e30 * u_pre - 1e30*theta))  (saturates to exact 0/1)
  Host upcasts to f32; norm rel-err from bf16 is ~1.7e-3, inside 2e-2.
"""

import sys

sys.path.insert(0, "/opt/trn_rl_repo")

import numpy as np

DT = 1.0
BETA = 15.0
THR = 0.25

B, F, L = 64, 256, 2048
N_CORES = 8
FB = 128          # partition block of F per core
NQ = 4            # time quarters
QLEN = L // NQ    # 512
TC = 16           # time-chunk length
W = 16            # warmup steps per segment
KD = 4            # DVE fused segment count
TD = 112          # DVE segment length
TP = 64           # Pool segment length
HB = 32           # half-batch for the two interleaved chain groups
SIGM = 1.0e30     # sigmoid saturation scale for the spike output

NDC = (W + TD) // TC   # 12 DVE chunks (3 warmup + 9 output)
NPC = (W + TP) // TC   # 8 Pool chunks (3 warmup + 5 output)
NWC = W // TC          # 3

_BUILD_CACHE: dict = {}
LAST_RESULTS = None  # BassKernelResults of the most recent kernel() call
_CURRENT_NC = None


def _get_current_nc():
    return _CURRENT_NC


def _build_v5():
    import concourse.bacc as bacc
    import concourse.mybir as mybir
    from concourse import tile

    f32 = mybir.dt.float32
    bf16 = mybir.dt.bfloat16
    fp8 = mybir.dt.float8e4
    Alu = mybir.AluOpType
    Act = mybir.ActivationFunctionType

    nc = bacc.Bacc(None, target_bir_lowering=False)

    iD = nc.dram_tensor("i_dve", [128, NDC, 2, TC, KD, HB], f32, kind="ExternalInput")
    iP = nc.dram_tensor("i_pool", [128, NPC, 2, TC, HB], f32, kind="ExternalInput")
    cst_d = nc.dram_tensor("consts", [128, 5], f32, kind="ExternalInput")

    nod, nop = NDC - NWC, NPC - NWC
    vD = nc.dram_tensor("v_dve", [128, nod, 2, TC, KD, HB], bf16, kind="ExternalOutput")
    zD = nc.dram_tensor("z_dve", [128, nod, 2, TC, KD, HB], bf16, kind="ExternalOutput")
    sD = nc.dram_tensor("s_dve", [128, nod, 2, TC, KD, HB], fp8, kind="ExternalOutput")
    vP = nc.dram_tensor("v_pool", [128, nop, 2, TC, HB], bf16, kind="ExternalOutput")
    zP = nc.dram_tensor("z_pool", [128, nop, 2, TC, HB], bf16, kind="ExternalOutput")
    sP = nc.dram_tensor("s_pool", [128, nop, 2, TC, HB], fp8, kind="ExternalOutput")

    with tile.TileContext(nc) as tc_:
        with (
            tc_.tile_pool(name="const", bufs=1) as cp,
            tc_.tile_pool(name="din", bufs=3) as din,
            tc_.tile_pool(name="dwork", bufs=4) as dw,
            tc_.tile_pool(name="dout", bufs=2) as do_,
            tc_.tile_pool(name="pin", bufs=5) as pin,
            tc_.tile_pool(name="pwork", bufs=3) as pw,
            tc_.tile_pool(name="pout", bufs=2) as po,
        ):
            # first input chunks, split fine so the chains can start early;
            # consts ride the ACT ring in one packed transfer.
            it0 = din.tile([128, 2, TC, KD, HB], f32, tag="di")
            for h in (0, 1):
                nc.sync.dma_start(it0[:, h, 0 : TC // 2], iD[:, 0, h, 0 : TC // 2])
            ip0 = pin.tile([128, 2, TC, HB], f32, tag="pi")
            for h in (0, 1):
                nc.sync.dma_start(ip0[:, h, 0 : TC // 2], iP[:, 0, h, 0 : TC // 2])
            for h in (0, 1):
                nc.sync.dma_start(it0[:, h, TC // 2 :], iD[:, 0, h, TC // 2 :])
                nc.sync.dma_start(ip0[:, h, TC // 2 :], iP[:, 0, h, TC // 2 :])

            cst_t = cp.tile([128, 5], f32, tag="cst")
            nc.scalar.dma_start(cst_t[:], cst_d[:])
            al_t = cst_t[:, 0:1]
            th_t = cst_t[:, 1:2]
            sv_t = cst_t[:, 2:3]
            sz_t = cst_t[:, 3:4]
            sb_t = cst_t[:, 4:5]

            ustD = cp.tile([128, 2, KD, HB], f32, tag="ustD")
            upwD = cp.tile([128, 2, KD, HB], f32, tag="upwD")
            nc.gpsimd.memset(ustD[:], 0.0)
            # Pool state holds u_pre (pre-reset); m/g are per-half scratch.
            upwP = cp.tile([128, 2, HB], f32, tag="upwP")
            mP = cp.tile([128, 2, HB], f32, tag="mP")
            gP = cp.tile([128, 2, HB], f32, tag="gP")
            nc.gpsimd.memset(upwP[:], 0.0)
            uprevP = [upwP[:, 0], upwP[:, 1]]
            pending_pout = []

            for k in range(NDC):
                # ---- DVE: KD fused segments, 2 interleaved b-half groups ----
                if k == 0:
                    it = it0
                else:
                    it = din.tile([128, 2, TC, KD, HB], f32, tag="di")
                    nc.sync.dma_start(it[:], iD[:, k])
                is_out = k >= NWC
                up = None
                if is_out:
                    up = dw.tile([128, 2, TC, KD, HB], f32, tag="dup")
                for t in range(TC):
                    if is_out:
                        dsts = (up[:, 0, t], up[:, 1, t])
                    else:
                        dsts = (upwD[:, 0], upwD[:, 1])
                    for h in (0, 1):
                        nc.vector.scalar_tensor_tensor(
                            dsts[h], ustD[:, h], al_t, it[:, h, t],
                            op0=Alu.mult, op1=Alu.add,
                        )
                    for h in (0, 1):
                        nc.vector.scalar_tensor_tensor(
                            ustD[:, h], dsts[h], th_t, dsts[h],
                            op0=Alu.is_lt, op1=Alu.mult,
                        )
                # deferred pool-chunk v/s on DVE (pool chunk k-1 is done)
                if pending_pout:
                    po_, pupp = pending_pout.pop(0)
                    pvt = po.tile([128, 2, TC, HB], bf16, tag="pv")
                    nc.vector.tensor_scalar(pvt[:], pupp[:], sv_t, None, Alu.mult)
                    pst = po.tile([128, 2, TC, HB], fp8, tag="ps")
                    nc.vector.tensor_scalar(pst[:], pupp[:], th_t, None, Alu.is_ge)
                    pzt = po.tile([128, 2, TC, HB], bf16, tag="pz")
                    nc.scalar.activation(pzt[:], pupp[:], Act.Copy, bias=-BETA * THR, scale=sz_t)
                    nc.scalar.dma_start(vP[:, po_], pvt[:])
                    nc.scalar.dma_start(zP[:, po_], pzt[:])
                    nc.scalar.dma_start(sP[:, po_], pst[:])

                if is_out and k == NDC - 1:
                    # tail chunk: per-half ops so out-DMAs overlap compute;
                    # v/s on the freed chain engine, z on ACT in parallel
                    o = k - NWC
                    vt = do_.tile([128, 2, TC, KD, HB], bf16, tag="dv")
                    zt = do_.tile([128, 2, TC, KD, HB], bf16, tag="dz")
                    st = do_.tile([128, 2, TC, KD, HB], fp8, tag="ds")
                    for h in (0, 1):
                        nc.vector.tensor_scalar(vt[:, h], up[:, h], sv_t, None, Alu.mult)
                        nc.scalar.activation(zt[:, h], up[:, h], Act.Copy, bias=-BETA * THR, scale=sz_t)
                        nc.vector.tensor_scalar(st[:, h], up[:, h], th_t, None, Alu.is_ge)
                        nc.scalar.dma_start(vD[:, o, h], vt[:, h])
                        nc.scalar.dma_start(zD[:, o, h], zt[:, h])
                        nc.scalar.dma_start(sD[:, o, h], st[:, h])
                elif is_out:
                    o = k - NWC
                    vt = do_.tile([128, 2, TC, KD, HB], bf16, tag="dv")
                    zt = do_.tile([128, 2, TC, KD, HB], bf16, tag="dz")
                    st = do_.tile([128, 2, TC, KD, HB], fp8, tag="ds")
                    if k == NWC:
                        # tail chunk (chain engines free) and first output
                        # chunk (ACT has no release credits yet): v/s on DVE
                        nc.vector.tensor_scalar(vt[:], up[:], sv_t, None, Alu.mult)
                        nc.vector.tensor_scalar(st[:], up[:], th_t, None, Alu.is_ge)
                    elif k >= NPC - 1:
                        # Pool chain is done by now; use it for v/s
                        nc.gpsimd.tensor_scalar(vt[:], up[:], sv_t, None, Alu.mult)
                        nc.gpsimd.tensor_scalar(st[:], up[:], th_t, None, Alu.is_ge)
                    else:
                        nc.scalar.activation(vt[:], up[:], Act.Copy, bias=0.0, scale=sv_t)
                        nc.scalar.activation(st[:], up[:], Act.Sigmoid, bias=sb_t, scale=SIGM)
                    nc.scalar.activation(zt[:], up[:], Act.Copy, bias=-BETA * THR, scale=sz_t)
                    nc.scalar.dma_start(vD[:, o], vt[:])
                    nc.scalar.dma_start(zD[:, o], zt[:])
                    nc.scalar.dma_start(sD[:, o], st[:])

                # ---- Pool: 1 segment, 3-op form, 2 interleaved b-half groups ----
                if k >= NPC:
                    continue
                if k == 0:
                    ip = ip0
                else:
                    ip = pin.tile([128, 2, TC, HB], f32, tag="pi")
                    nc.sync.dma_start(ip[:], iP[:, k])
                is_out = k >= NWC
                upp = None
                if is_out:
                    upp = pw.tile([128, 2, TC, HB], f32, tag="pup")
                for t in range(TC):
                    if is_out:
                        dsts = (upp[:, 0, t], upp[:, 1, t])
                    else:
                        dsts = (upwP[:, 0], upwP[:, 1])
                    for h in (0, 1):
                        nc.gpsimd.tensor_scalar(
                            mP[:, h], uprevP[h], th_t, al_t,
                            Alu.is_lt, Alu.mult,
                        )
                    for h in (0, 1):
                        nc.gpsimd.tensor_mul(gP[:, h], uprevP[h], mP[:, h])
                    for h in (0, 1):
                        nc.gpsimd.tensor_add(dsts[h], gP[:, h], ip[:, h, t])
                    uprevP = [dsts[0], dsts[1]]
                if is_out:
                    pending_pout.append((k - NWC, upp))

            # flush remaining pool outputs (after all chains)
            for po_, pupp in pending_pout:
                pvt = po.tile([128, 2, TC, HB], bf16, tag="pv")
                nc.vector.tensor_scalar(pvt[:], pupp[:], sv_t, None, Alu.mult)
                pst = po.tile([128, 2, TC, HB], fp8, tag="ps")
                nc.vector.tensor_scalar(pst[:], pupp[:], th_t, None, Alu.is_ge)
                pzt = po.tile([128, 2, TC, HB], bf16, tag="pz")
                nc.scalar.activation(pzt[:], pupp[:], Act.Copy, bias=-BETA * THR, scale=sz_t)
                nc.scalar.dma_start(vP[:, po_], pvt[:])
                nc.scalar.dma_start(zP[:, po_], pzt[:])
                nc.scalar.dma_start(sP[:, po_], pst[:])

    nc.compile()
    return nc


def _get_nc():
    key = ("v5", TC, W, KD, TD, TP)
    if key not in _BUILD_CACHE:
        _BUILD_CACHE[key] = _build_v5()
    return _BUILD_CACHE[key]


# --------------------------------------------------------------------------
# Host side


def _alpha_host(raw_tau: np.ndarray) -> tuple[np.ndarray, np.ndarray]:
    """alpha = exp(-DT / (softplus(raw_tau) + 1e-4)) with the same jax ops /
    device as the reference, so spike threshold comparisons match closely."""
    import jax
    import jax.numpy as jnp

    with jax.default_device(jax.devices("cpu")[0]):
        tau = jax.nn.softplus(jnp.asarray(np.asarray(raw_tau))) + 1e-4
        alpha = np.asarray(jnp.exp(-DT / tau), dtype=np.float32)
    one_minus = (np.float32(1.0) - alpha).astype(np.float32)
    return alpha, one_minus


def _pack_core(IT: np.ndarray, q: int) -> tuple[np.ndarray, np.ndarray]:
    """IT: [128, L, 64] (f, t, b) for this core's f-block. Returns the DVE
    and Pool input streams with warmup prefixes (zero-padded below t=0)."""
    r0 = q * QLEN
    tD = np.empty((W + TD, KD), np.int64)
    for j in range(KD):
        start = r0 + j * TD
        tD[:, j] = np.arange(start - W, start + TD)
    tP = np.arange(r0 + KD * TD - W, r0 + QLEN)

    mD = tD >= 0
    mP = tP >= 0
    iD = IT[:, np.clip(tD, 0, L - 1), :]        # [128, W+TD, KD, 64]
    iD[:, ~mD] = 0.0
    iP = IT[:, np.clip(tP, 0, L - 1), :]        # [128, W+TP, 64]
    iP[:, ~mP] = 0.0
    # device layout: [128, chunk, h(2), t(TC), KD, HB] with b = h*HB + b'
    iD = iD.reshape(128, NDC, TC, KD, 2, HB).transpose(0, 1, 4, 2, 3, 5)
    iP = iP.reshape(128, NPC, TC, 2, HB).transpose(0, 1, 3, 2, 4)
    return np.ascontiguousarray(iD), np.ascontiguousarray(iP)


def kernel(I: np.ndarray, raw_tau: np.ndarray, _trace: bool = False):
    global LAST_RESULTS, _CURRENT_NC
    from concourse.bass_utils import run_bass_kernel_spmd

    I = np.asarray(I, dtype=np.float32)
    raw_tau = np.asarray(raw_tau, dtype=np.float32)
    assert I.shape == (B, F, L), I.shape

    alpha, om = _alpha_host(raw_tau)
    theta = (np.float32(THR) / om).astype(np.float32)
    sc_z = (np.float32(BETA) * om).astype(np.float32)
    sg_b = (-np.float32(SIGM) * theta).astype(np.float32)

    nc = _get_nc()
    _CURRENT_NC = nc

    in_maps = []
    for c in range(N_CORES):
        fb, q = c % 2, c // 2
        fsl = slice(fb * FB, (fb + 1) * FB)
        IT = np.ascontiguousarray(I[:, fsl, :].transpose(1, 2, 0))  # [128, L, 64]
        iD, iP = _pack_core(IT, q)
        cst = np.stack(
            [alpha[fsl], theta[fsl], om[fsl], sc_z[fsl], sg_b[fsl]], axis=1
        ).astype(np.float32)
        in_maps.append(
            {"i_dve": iD, "i_pool": iP, "consts": np.ascontiguousarray(cst)}
        )

    res = run_bass_kernel_spmd(nc, in_maps, core_ids=list(range(N_CORES)), trace=_trace)
    LAST_RESULTS = res

    v = np.empty((B, F, L), np.float32)
    z = np.empty((B, F, L), np.float32)
    s = np.empty((B, F, L), np.float32)
    for c in range(N_CORES):
        fb, q = c % 2, c // 2
        fsl = slice(fb * FB, (fb + 1) * FB)
        r0 = q * QLEN
        r = res.results[c]
        nod = TD // TC
        for name, dst in (("v_dve", v), ("z_dve", z), ("s_dve", s)):
            a = np.asarray(r[name]).astype(np.float32).reshape(128, nod, 2, TC, KD, HB)
            a = a.transpose(0, 1, 3, 4, 2, 5).reshape(128, TD, KD, 64)
            for j in range(KD):
                t0 = r0 + j * TD
                dst[:, fsl, t0 : t0 + TD] = a[:, :, j, :].transpose(2, 0, 1)
        nop = TP // TC
        for name, dst in (("v_pool", v), ("z_pool", z), ("s_pool", s)):
            a = np.asarray(r[name]).astype(np.float32).reshape(128, nop, 2, TC, HB)
            a = a.transpose(0, 1, 3, 2, 4).reshape(128, TP, 64)
            t0 = r0 + KD * TD
            dst[:, fsl, t0 : t0 + TP] = a.transpose(2, 0, 1)
    return v, z, s
